# revision 4
# baseline (speedup 1.0000x reference)
"""Trainium2 Bass kernel for nn_AdaptiveAnchorGAT.

Math note: in the reference FCA, score[i,j] = t_i.a1 + t_j.a2, so the
row-constant t_i.a1 cancels inside the row softmax -> every row of the
attention output equals the same softmax(t.a2)-weighted mean of t (rank-1).
The second FCA's input rows are then all identical, so its output is just
t2 = LN(sent @ anchors.T) @ recv_W broadcast over the batch.  This collapses
the B x B attention to O(B*F) work and is exact in real arithmetic
(verified <1.3e-6 max-rel vs the jax reference).

Distribution: the user-batch pipeline (gather, LN, t1, softmax-weighted sum,
anchor projection, LN2, t2, sin residual) is tiny and is computed redundantly
on all 8 cores.  The dominant cost, preds = ue @ item_table.T
([4096,128]x[128,50000], 819 MB f32 output), is sharded over items: core c
computes preds[:, c*6250:(c+1)*6250].  No collectives needed.
"""

import sys

sys.path.insert(0, "/opt/trn_rl_repo")

import numpy as np

import concourse.bacc as bacc
import concourse.bass as bass
import concourse.tile as tile
from concourse import mybir
from concourse.bass_utils import run_bass_kernel_spmd
from concourse.masks import make_identity

B = 4096       # batch of users
D = 128        # embedding dim
NA = 128       # num anchors
AD = 128       # anchor dim
NU = 100000    # num users
NI = 50000     # num items
NCORES = 8
NI_SH = NI // NCORES   # 6250 items per core
NT = B // 128          # 32 user tiles
F32 = mybir.dt.float32
I32 = mybir.dt.int32

# preds free-dim chunking: 12 x 512 + 106
N_FULL = NI_SH // 512          # 12
N_REM = NI_SH - N_FULL * 512   # 106


def build_nc():
    nc = bacc.Bacc(None, target_bir_lowering=False)

    # ---- DRAM parameters (per-core shards prepared on host) ----
    uidx = nc.declare_dram_parameter("uidx", [128, NT], I32, isOutput=False)          # user idx, [p, j] = idx[j*128+p]
    pidx = nc.declare_dram_parameter("pidx", [128, NT], I32, isOutput=False)          # pos item idx, same layout
    user_table = nc.declare_dram_parameter("user_table", [NU, D], F32, isOutput=False)
    item_table = nc.declare_dram_parameter("item_table", [NI, D], F32, isOutput=False)
    item_t = nc.declare_dram_parameter("item_t", [D, NI_SH], F32, isOutput=False)     # item_table.T shard
    send_W = nc.declare_dram_parameter("send_W", [D, AD], F32, isOutput=False)
    a2_b = nc.declare_dram_parameter("a2_b", [128, AD], F32, isOutput=False)          # send_a[AD:] tiled over partitions
    anchors_T = nc.declare_dram_parameter("anchors_T", [AD, NA], F32, isOutput=False)
    recv_W = nc.declare_dram_parameter("recv_W", [NA, D], F32, isOutput=False)
    ln1_g = nc.declare_dram_parameter("ln1_g", [128, D], F32, isOutput=False)         # tiled over partitions
    ln1_b = nc.declare_dram_parameter("ln1_b", [128, D], F32, isOutput=False)
    ln2_g = nc.declare_dram_parameter("ln2_g", [1, NA], F32, isOutput=False)
    ln2_b = nc.declare_dram_parameter("ln2_b", [1, NA], F32, isOutput=False)

    preds = nc.declare_dram_parameter("preds", [B, NI_SH], F32, isOutput=True)
    ue_out = nc.declare_dram_parameter("ue", [B, D], F32, isOutput=True)
    pos_out = nc.declare_dram_parameter("pos", [B, D], F32, isOutput=True)

    with tile.TileContext(nc) as tc:
        with (
            tc.tile_pool(name="consts", bufs=1) as consts,
            tc.tile_pool(name="big", bufs=1) as big,
            tc.tile_pool(name="work", bufs=4) as work,
            tc.tile_pool(name="outst", bufs=2) as outst,
            tc.tile_pool(name="pp", bufs=4, space="PSUM") as pp,
            tc.tile_pool(name="acc", bufs=1, space="PSUM") as acc,
        ):
            # ---- constants into SBUF ----
            ident = consts.tile([128, 128], F32)
            make_identity(nc, ident[:])
            eps = consts.tile([128, 1], F32)
            nc.vector.memset(eps[:], 1e-5)
            ones_row = consts.tile([1, 128], F32)
            nc.vector.memset(ones_row[:], 1.0)
            ones_col = consts.tile([128, 1], F32)
            nc.vector.memset(ones_col[:], 1.0)

            uidx_sb = consts.tile([128, NT], I32)
            nc.sync.dma_start(out=uidx_sb[:], in_=uidx[:])
            pidx_sb = consts.tile([128, NT], I32)
            nc.sync.dma_start(out=pidx_sb[:], in_=pidx[:])
            send_W_sb = consts.tile([D, AD], F32)
            nc.sync.dma_start(out=send_W_sb[:], in_=send_W[:])
            a2_sb = consts.tile([128, AD], F32)
            nc.sync.dma_start(out=a2_sb[:], in_=a2_b[:])
            anchors_T_sb = consts.tile([AD, NA], F32)
            nc.sync.dma_start(out=anchors_T_sb[:], in_=anchors_T[:])
            recv_W_sb = consts.tile([NA, D], F32)
            nc.sync.dma_start(out=recv_W_sb[:], in_=recv_W[:])
            ln1_g_sb = consts.tile([128, D], F32)
            nc.sync.dma_start(out=ln1_g_sb[:], in_=ln1_g[:])
            ln1_b_sb = consts.tile([128, D], F32)
            nc.sync.dma_start(out=ln1_b_sb[:], in_=ln1_b[:])
            ln2_g_sb = consts.tile([1, NA], F32)
            nc.sync.dma_start(out=ln2_g_sb[:], in_=ln2_g[:])
            ln2_b_sb = consts.tile([1, NA], F32)
            nc.sync.dma_start(out=ln2_b_sb[:], in_=ln2_b[:])

            # item_t fully resident in SBUF (3.2 MB)
            item_t_sb = big.tile([D, NI_SH], F32)
            nc.sync.dma_start(out=item_t_sb[:], in_=item_t[:])

            # persistent big tiles
            u_all = big.tile([128, B], F32)      # u (later ue), tile j at cols j*128..
            t1_all = big.tile([128, B], F32)     # t1 = LN1(u) @ send_W
            ueT_all = big.tile([128, B], F32)    # ue transposed per tile: [D, users]
            s_all = big.tile([128, NT], F32)     # pre-softmax scores, [p, j] = s[j*128+p]

            # ---- pre-phase: per user tile ----
            for j in range(NT):
                js = slice(j * 128, (j + 1) * 128)
                # gather user rows
                nc.gpsimd.indirect_dma_start(
                    out=u_all[:, js],
                    out_offset=None,
                    in_=user_table[:],
                    in_offset=bass.IndirectOffsetOnAxis(ap=uidx_sb[:, j : j + 1], axis=0),
                )
                # LN1 rowwise
                stats = work.tile([128, 6], F32, tag="stats")
                nc.vector.bn_stats(out=stats[:], in_=u_all[:, js])
                mv = work.tile([128, 2], F32, tag="mv")
                nc.vector.bn_aggr(out=mv[:], in_=stats[:])
                nc.scalar.activation(
                    out=mv[:, 1:2], in_=mv[:, 1:2],
                    func=mybir.ActivationFunctionType.Sqrt, bias=eps[:],
                )
                nc.vector.reciprocal(out=mv[:, 1:2], in_=mv[:, 1:2])
                nf = work.tile([128, D], F32, tag="nf")
                nc.vector.tensor_scalar(
                    out=nf[:], in0=u_all[:, js],
                    scalar1=mv[:, 0:1], scalar2=mv[:, 1:2],
                    op0=mybir.AluOpType.subtract, op1=mybir.AluOpType.mult,
                )
                nc.vector.tensor_mul(out=nf[:], in0=nf[:], in1=ln1_g_sb[:])
                nc.vector.tensor_add(out=nf[:], in0=nf[:], in1=ln1_b_sb[:])
                # nf_T
                nfT_ps = pp.tile([128, 128], F32, tag="pp")
                nc.tensor.transpose(out=nfT_ps[:], in_=nf[:], identity=ident[:])
                nfT = work.tile([128, 128], F32, tag="nfT")
                nc.vector.tensor_copy(out=nfT[:], in_=nfT_ps[:])
                # t1 = nf @ send_W  -> [users, AD]
                t1_ps = pp.tile([128, AD], F32, tag="pp")
                nc.tensor.matmul(out=t1_ps[:], lhsT=nfT[:], rhs=send_W_sb[:], start=True, stop=True)
                nc.vector.tensor_copy(out=t1_all[:, js], in_=t1_ps[:])
                # s = t1 . a2 (rowwise)
                sa = work.tile([128, AD], F32, tag="sa")
                nc.vector.tensor_mul(out=sa[:], in0=t1_all[:, js], in1=a2_sb[:])
                nc.vector.reduce_sum(out=s_all[:, j : j + 1], in_=sa[:], axis=mybir.AxisListType.X)

            # ---- softmax weights over the whole batch (no max-sub; scores are tiny) ----
            w_all = big.tile([128, NT], F32)
            nc.scalar.activation(out=w_all[:], in_=s_all[:], func=mybir.ActivationFunctionType.Exp)

            # sent_unnorm[ad] = sum_u t1[u, ad] * w[u]   (32 accumulating matmuls)
            sent_ps = acc.tile([AD, 1], F32, tag="sent")
            for j in range(NT):
                js = slice(j * 128, (j + 1) * 128)
                nc.tensor.matmul(
                    out=sent_ps[:], lhsT=t1_all[:, js], rhs=w_all[:, j : j + 1],
                    start=(j == 0), stop=(j == NT - 1),
                )
            sent_sb = work.tile([AD, 1], F32, tag="sent_sb")
            nc.vector.tensor_copy(out=sent_sb[:], in_=sent_ps[:])

            # denom = sum(w)
            wsum = work.tile([128, 1], F32, tag="wsum")
            nc.vector.reduce_sum(out=wsum[:], in_=w_all[:], axis=mybir.AxisListType.X)
            den_ps = acc.tile([1, 1], F32, tag="den")
            nc.tensor.matmul(out=den_ps[:], lhsT=wsum[:], rhs=ones_col[:], start=True, stop=True)
            inv_den = work.tile([1, 1], F32, tag="invden")
            nc.vector.reciprocal(out=inv_den[:], in_=den_ps[:])

            # ap_row = (sent_unnorm @ anchors.T) / denom   [1, NA]
            ap_ps = acc.tile([1, NA], F32, tag="ppn")
            nc.tensor.matmul(out=ap_ps[:], lhsT=sent_sb[:], rhs=anchors_T_sb[:], start=True, stop=True)
            ap_row = work.tile([1, NA], F32, tag="ap_row")
            nc.vector.tensor_scalar_mul(out=ap_row[:], in0=ap_ps[:], scalar1=inv_den[:])

            # LN2 on the [1, NA] row
            st2 = work.tile([1, 6], F32, tag="st2")
            nc.vector.bn_stats(out=st2[:], in_=ap_row[:])
            mv2 = work.tile([1, 2], F32, tag="mv2")
            nc.vector.bn_aggr(out=mv2[:], in_=st2[:])
            nc.scalar.activation(
                out=mv2[:, 1:2], in_=mv2[:, 1:2],
                func=mybir.ActivationFunctionType.Sqrt, bias=eps[:1],
            )
            nc.vector.reciprocal(out=mv2[:, 1:2], in_=mv2[:, 1:2])
            na_row = work.tile([1, NA], F32, tag="na_row")
            nc.vector.tensor_scalar(
                out=na_row[:], in0=ap_row[:],
                scalar1=mv2[:, 0:1], scalar2=mv2[:, 1:2],
                op0=mybir.AluOpType.subtract, op1=mybir.AluOpType.mult,
            )
            nc.vector.tensor_mul(out=na_row[:], in0=na_row[:], in1=ln2_g_sb[:])
            nc.vector.tensor_add(out=na_row[:], in0=na_row[:], in1=ln2_b_sb[:])

            # na as column, then t2_row = na @ recv_W  [1, D]
            naT_ps = acc.tile([NA, 1], F32, tag="ppn")
            nc.tensor.transpose(out=naT_ps[:], in_=na_row[:], identity=ident[:1, :1])
            na_col = work.tile([NA, 1], F32, tag="na_col")
            nc.vector.tensor_copy(out=na_col[:], in_=naT_ps[:])
            t2_ps = acc.tile([1, D], F32, tag="ppn")
            nc.tensor.matmul(out=t2_ps[:], lhsT=na_col[:], rhs=recv_W_sb[:], start=True, stop=True)
            sin_row = work.tile([1, D], F32, tag="sin_row")
            nc.scalar.activation(out=sin_row[:], in_=t2_ps[:], func=mybir.ActivationFunctionType.Sin)

            # broadcast sin(t2) to all partitions via outer product with ones
            sinb_ps = pp.tile([128, D], F32, tag="pp")
            nc.tensor.matmul(out=sinb_ps[:], lhsT=ones_row[:], rhs=sin_row[:], start=True, stop=True)
            sinb = work.tile([128, D], F32, tag="sinb")
            nc.vector.tensor_copy(out=sinb[:], in_=sinb_ps[:])

            # ---- ue = u + sin(t2); write ue; build ueT; preds ----
            for j in range(NT):
                js = slice(j * 128, (j + 1) * 128)
                nc.vector.tensor_add(out=u_all[:, js], in0=u_all[:, js], in1=sinb[:])
                nc.sync.dma_start(out=ue_out[js, :], in_=u_all[:, js])
                ueT_ps = pp.tile([128, 128], F32, tag="pp")
                nc.tensor.transpose(out=ueT_ps[:], in_=u_all[:, js], identity=ident[:])
                nc.vector.tensor_copy(out=ueT_all[:, js], in_=ueT_ps[:])

            # pos gather (independent)
            for j in range(NT):
                js = slice(j * 128, (j + 1) * 128)
                pos_t = work.tile([128, D], F32, tag="pos")
                nc.gpsimd.indirect_dma_start(
                    out=pos_t[:],
                    out_offset=None,
                    in_=item_table[:],
                    in_offset=bass.IndirectOffsetOnAxis(ap=pidx_sb[:, j : j + 1], axis=0),
                )
                nc.sync.dma_start(out=pos_out[js, :], in_=pos_t[:])

            # preds[j*128:(j+1)*128, :] = ue_tile @ item_t shard
            for j in range(NT):
                js = slice(j * 128, (j + 1) * 128)
                orow = outst.tile([128, NI_SH], F32, tag="orow")
                for c in range(N_FULL):
                    cs = slice(c * 512, (c + 1) * 512)
                    p_ps = pp.tile([128, 512], F32, tag="pp")
                    nc.tensor.matmul(
                        out=p_ps[:], lhsT=ueT_all[:, js], rhs=item_t_sb[:, cs],
                        start=True, stop=True,
                    )
                    nc.vector.tensor_copy(out=orow[:, cs], in_=p_ps[:])
                rs = slice(N_FULL * 512, NI_SH)
                p_ps = pp.tile([128, 512], F32, tag="pp")
                nc.tensor.matmul(
                    out=p_ps[:, :N_REM], lhsT=ueT_all[:, js], rhs=item_t_sb[:, rs],
                    start=True, stop=True,
                )
                nc.vector.tensor_copy(out=orow[:, rs], in_=p_ps[:, :N_REM])
                nc.sync.dma_start(out=preds[js, :], in_=orow[:])

    nc.finalize()
    return nc


_NC = None


def _get_nc():
    global _NC
    if _NC is None:
        _NC = build_nc()
    return _NC


def _prep_in_maps(inputs):
    inp = {k: np.asarray(v) for k, v in inputs.items()}
    uidx = np.ascontiguousarray(
        inp["user_indices"].astype(np.int32).reshape(NT, 128).T
    )
    pidx = np.ascontiguousarray(
        inp["pos_item_indices"].astype(np.int32).reshape(NT, 128).T
    )
    user_table = np.ascontiguousarray(inp["user_table"], dtype=np.float32)
    item_table = np.ascontiguousarray(inp["item_table"], dtype=np.float32)
    item_T = np.ascontiguousarray(item_table.T)  # [D, NI]
    send_W = np.ascontiguousarray(inp["send_W"], dtype=np.float32)
    a2 = np.asarray(inp["send_a"], dtype=np.float32)[AD:]
    a2_b = np.ascontiguousarray(np.tile(a2[None, :], (128, 1)))
    anchors_T = np.ascontiguousarray(np.asarray(inp["anchors"], dtype=np.float32).T)
    recv_W = np.ascontiguousarray(inp["recv_W"], dtype=np.float32)
    ln1_g = np.ascontiguousarray(np.tile(np.asarray(inp["ln1_g"], np.float32)[None, :], (128, 1)))
    ln1_b = np.ascontiguousarray(np.tile(np.asarray(inp["ln1_b"], np.float32)[None, :], (128, 1)))
    ln2_g = np.asarray(inp["ln2_g"], np.float32)[None, :]
    ln2_b = np.asarray(inp["ln2_b"], np.float32)[None, :]

    common = dict(
        uidx=uidx, pidx=pidx, user_table=user_table, item_table=item_table,
        send_W=send_W, a2_b=a2_b, anchors_T=anchors_T, recv_W=recv_W,
        ln1_g=ln1_g, ln1_b=ln1_b, ln2_g=ln2_g, ln2_b=ln2_b,
    )
    in_maps = []
    for c in range(NCORES):
        m = dict(common)
        m["item_t"] = np.ascontiguousarray(item_T[:, c * NI_SH : (c + 1) * NI_SH])
        in_maps.append(m)
    return in_maps


def kernel(**inputs):
    nc = _get_nc()
    in_maps = _prep_in_maps(inputs)
    res = run_bass_kernel_spmd(nc, in_maps, core_ids=list(range(NCORES)))
    preds = np.concatenate([res.results[c]["preds"] for c in range(NCORES)], axis=1)
    ue = res.results[0]["ue"]
    pos = res.results[0]["pos"]
    return preds, ue, pos


# revision 5
# speedup vs baseline: 1.0403x; 1.0403x over previous
"""Trainium2 Bass kernel for nn_AdaptiveAnchorGAT.

Math note: in the reference FCA, score[i,j] = t_i.a1 + t_j.a2, so the
row-constant t_i.a1 cancels inside the row softmax -> every row of the
attention output equals the same softmax(t.a2)-weighted mean of t (rank-1).
The second FCA's input rows are then all identical, so its output is just
t2 = LN(sent @ anchors.T) @ recv_W broadcast over the batch.  This collapses
the B x B attention to O(B*F) work and is exact in real arithmetic
(verified <1.3e-6 max-rel vs the jax reference).

Distribution: the user-batch pipeline (gather, LN, t1, softmax-weighted sum,
anchor projection, LN2, t2, sin residual) is tiny and is computed redundantly
on all 8 cores.  The dominant cost, preds = ue @ item_table.T
([4096,128]x[128,50000], 819 MB f32 output), is sharded over items: core c
computes preds[:, c*6250:(c+1)*6250].  No collectives needed.
"""

import sys

sys.path.insert(0, "/opt/trn_rl_repo")

import numpy as np

import concourse.bacc as bacc
import concourse.bass as bass
import concourse.tile as tile
from concourse import mybir
from concourse.bass_utils import run_bass_kernel_spmd
from concourse.masks import make_identity

B = 4096       # batch of users
D = 128        # embedding dim
NA = 128       # num anchors
AD = 128       # anchor dim
NU = 100000    # num users
NI = 50000     # num items
NCORES = 8
NI_SH = NI // NCORES   # 6250 items per core
NT = B // 128          # 32 user tiles
F32 = mybir.dt.float32
BF16 = mybir.dt.bfloat16
I32 = mybir.dt.int32
COMPUTE_BF16 = True   # bf16 operands for the big preds matmul (PSUM accum stays f32)
MM_DT = BF16 if COMPUTE_BF16 else F32

# preds free-dim chunking: 12 x 512 + 106
N_FULL = NI_SH // 512          # 12
N_REM = NI_SH - N_FULL * 512   # 106


def build_nc():
    nc = bacc.Bacc(None, target_bir_lowering=False)

    # ---- DRAM parameters (per-core shards prepared on host) ----
    uidx = nc.declare_dram_parameter("uidx", [128, NT], I32, isOutput=False)          # user idx, [p, j] = idx[j*128+p]
    pidx = nc.declare_dram_parameter("pidx", [128, NT], I32, isOutput=False)          # pos item idx, same layout
    user_table = nc.declare_dram_parameter("user_table", [NU, D], F32, isOutput=False)
    item_table = nc.declare_dram_parameter("item_table", [NI, D], F32, isOutput=False)
    item_t = nc.declare_dram_parameter("item_t", [D, NI_SH], MM_DT, isOutput=False)     # item_table.T shard
    send_W = nc.declare_dram_parameter("send_W", [D, AD], F32, isOutput=False)
    a2_b = nc.declare_dram_parameter("a2_b", [128, AD], F32, isOutput=False)          # send_a[AD:] tiled over partitions
    anchors_T = nc.declare_dram_parameter("anchors_T", [AD, NA], F32, isOutput=False)
    recv_W = nc.declare_dram_parameter("recv_W", [NA, D], F32, isOutput=False)
    ln1_g = nc.declare_dram_parameter("ln1_g", [128, D], F32, isOutput=False)         # tiled over partitions
    ln1_b = nc.declare_dram_parameter("ln1_b", [128, D], F32, isOutput=False)
    ln2_g = nc.declare_dram_parameter("ln2_g", [1, NA], F32, isOutput=False)
    ln2_b = nc.declare_dram_parameter("ln2_b", [1, NA], F32, isOutput=False)

    preds = nc.declare_dram_parameter("preds", [B, NI_SH], F32, isOutput=True)
    ue_out = nc.declare_dram_parameter("ue", [B, D], F32, isOutput=True)
    pos_out = nc.declare_dram_parameter("pos", [B, D], F32, isOutput=True)

    with tile.TileContext(nc) as tc:
        with (
            tc.tile_pool(name="consts", bufs=1) as consts,
            tc.tile_pool(name="big", bufs=1) as big,
            tc.tile_pool(name="work", bufs=4) as work,
            tc.tile_pool(name="outst", bufs=2) as outst,
            tc.tile_pool(name="pp", bufs=4, space="PSUM") as pp,
            tc.tile_pool(name="acc", bufs=1, space="PSUM") as acc,
        ):
            # ---- constants into SBUF ----
            ident = consts.tile([128, 128], F32)
            make_identity(nc, ident[:])
            eps = consts.tile([128, 1], F32)
            nc.vector.memset(eps[:], 1e-5)
            ones_row = consts.tile([1, 128], F32)
            nc.vector.memset(ones_row[:], 1.0)
            ones_col = consts.tile([128, 1], F32)
            nc.vector.memset(ones_col[:], 1.0)

            uidx_sb = consts.tile([128, NT], I32)
            nc.sync.dma_start(out=uidx_sb[:], in_=uidx[:])
            pidx_sb = consts.tile([128, NT], I32)
            nc.sync.dma_start(out=pidx_sb[:], in_=pidx[:])
            send_W_sb = consts.tile([D, AD], F32)
            nc.sync.dma_start(out=send_W_sb[:], in_=send_W[:])
            a2_sb = consts.tile([128, AD], F32)
            nc.sync.dma_start(out=a2_sb[:], in_=a2_b[:])
            anchors_T_sb = consts.tile([AD, NA], F32)
            nc.sync.dma_start(out=anchors_T_sb[:], in_=anchors_T[:])
            recv_W_sb = consts.tile([NA, D], F32)
            nc.sync.dma_start(out=recv_W_sb[:], in_=recv_W[:])
            ln1_g_sb = consts.tile([128, D], F32)
            nc.sync.dma_start(out=ln1_g_sb[:], in_=ln1_g[:])
            ln1_b_sb = consts.tile([128, D], F32)
            nc.sync.dma_start(out=ln1_b_sb[:], in_=ln1_b[:])
            ln2_g_sb = consts.tile([1, NA], F32)
            nc.sync.dma_start(out=ln2_g_sb[:], in_=ln2_g[:])
            ln2_b_sb = consts.tile([1, NA], F32)
            nc.sync.dma_start(out=ln2_b_sb[:], in_=ln2_b[:])

            # item_t fully resident in SBUF (3.2 MB)
            item_t_sb = big.tile([D, NI_SH], MM_DT)
            nc.sync.dma_start(out=item_t_sb[:], in_=item_t[:])

            # persistent big tiles
            u_all = big.tile([128, B], F32)      # u (later ue), tile j at cols j*128..
            t1_all = big.tile([128, B], F32)     # t1 = LN1(u) @ send_W
            ueT_all = big.tile([128, B], MM_DT)  # ue transposed per tile: [D, users]
            s_all = big.tile([128, NT], F32)     # pre-softmax scores, [p, j] = s[j*128+p]

            w_all = big.tile([128, NT], F32)
            sent_ps = acc.tile([AD, 1], F32, tag="sent")

            # ---- pre-phase: per user tile ----
            for j in range(NT):
                js = slice(j * 128, (j + 1) * 128)
                # gather user rows
                nc.gpsimd.indirect_dma_start(
                    out=u_all[:, js],
                    out_offset=None,
                    in_=user_table[:],
                    in_offset=bass.IndirectOffsetOnAxis(ap=uidx_sb[:, j : j + 1], axis=0),
                )
                # LN1 rowwise
                stats = work.tile([128, 6], F32, tag="stats")
                nc.vector.bn_stats(out=stats[:], in_=u_all[:, js])
                mv = work.tile([128, 2], F32, tag="mv")
                nc.vector.bn_aggr(out=mv[:], in_=stats[:])
                nc.scalar.activation(
                    out=mv[:, 1:2], in_=mv[:, 1:2],
                    func=mybir.ActivationFunctionType.Sqrt, bias=eps[:],
                )
                nc.vector.reciprocal(out=mv[:, 1:2], in_=mv[:, 1:2])
                nf = work.tile([128, D], F32, tag="nf")
                nc.vector.tensor_scalar(
                    out=nf[:], in0=u_all[:, js],
                    scalar1=mv[:, 0:1], scalar2=mv[:, 1:2],
                    op0=mybir.AluOpType.subtract, op1=mybir.AluOpType.mult,
                )
                nc.vector.tensor_mul(out=nf[:], in0=nf[:], in1=ln1_g_sb[:])
                nc.vector.tensor_add(out=nf[:], in0=nf[:], in1=ln1_b_sb[:])
                # nf_T
                nfT_ps = pp.tile([128, 128], F32, tag="pp")
                nc.tensor.transpose(out=nfT_ps[:], in_=nf[:], identity=ident[:])
                nfT = work.tile([128, 128], F32, tag="nfT")
                nc.vector.tensor_copy(out=nfT[:], in_=nfT_ps[:])
                # t1 = nf @ send_W  -> [users, AD]
                t1_ps = pp.tile([128, AD], F32, tag="pp")
                nc.tensor.matmul(out=t1_ps[:], lhsT=nfT[:], rhs=send_W_sb[:], start=True, stop=True)
                nc.vector.tensor_copy(out=t1_all[:, js], in_=t1_ps[:])
                # s = t1 . a2 (rowwise), w = exp(s), accumulate sent += t1.T @ w
                sa = work.tile([128, AD], F32, tag="sa")
                nc.vector.tensor_mul(out=sa[:], in0=t1_all[:, js], in1=a2_sb[:])
                nc.vector.reduce_sum(out=s_all[:, j : j + 1], in_=sa[:], axis=mybir.AxisListType.X)
                nc.scalar.activation(
                    out=w_all[:, j : j + 1], in_=s_all[:, j : j + 1],
                    func=mybir.ActivationFunctionType.Exp,
                )
                nc.tensor.matmul(
                    out=sent_ps[:], lhsT=t1_all[:, js], rhs=w_all[:, j : j + 1],
                    start=(j == 0), stop=(j == NT - 1), skip_group_check=True,
                )

            sent_sb = work.tile([AD, 1], F32, tag="sent_sb")
            nc.vector.tensor_copy(out=sent_sb[:], in_=sent_ps[:])

            # denom = sum(w)
            wsum = work.tile([128, 1], F32, tag="wsum")
            nc.vector.reduce_sum(out=wsum[:], in_=w_all[:], axis=mybir.AxisListType.X)
            den_ps = acc.tile([1, 1], F32, tag="den")
            nc.tensor.matmul(out=den_ps[:], lhsT=wsum[:], rhs=ones_col[:], start=True, stop=True)
            inv_den = work.tile([1, 1], F32, tag="invden")
            nc.vector.reciprocal(out=inv_den[:], in_=den_ps[:])

            # ap_row = (sent_unnorm @ anchors.T) / denom   [1, NA]
            ap_ps = acc.tile([1, NA], F32, tag="ppn")
            nc.tensor.matmul(out=ap_ps[:], lhsT=sent_sb[:], rhs=anchors_T_sb[:], start=True, stop=True)
            ap_row = work.tile([1, NA], F32, tag="ap_row")
            nc.vector.tensor_scalar_mul(out=ap_row[:], in0=ap_ps[:], scalar1=inv_den[:])

            # LN2 on the [1, NA] row
            st2 = work.tile([1, 6], F32, tag="st2")
            nc.vector.bn_stats(out=st2[:], in_=ap_row[:])
            mv2 = work.tile([1, 2], F32, tag="mv2")
            nc.vector.bn_aggr(out=mv2[:], in_=st2[:])
            nc.scalar.activation(
                out=mv2[:, 1:2], in_=mv2[:, 1:2],
                func=mybir.ActivationFunctionType.Sqrt, bias=eps[:1],
            )
            nc.vector.reciprocal(out=mv2[:, 1:2], in_=mv2[:, 1:2])
            na_row = work.tile([1, NA], F32, tag="na_row")
            nc.vector.tensor_scalar(
                out=na_row[:], in0=ap_row[:],
                scalar1=mv2[:, 0:1], scalar2=mv2[:, 1:2],
                op0=mybir.AluOpType.subtract, op1=mybir.AluOpType.mult,
            )
            nc.vector.tensor_mul(out=na_row[:], in0=na_row[:], in1=ln2_g_sb[:])
            nc.vector.tensor_add(out=na_row[:], in0=na_row[:], in1=ln2_b_sb[:])

            # na as column, then t2_row = na @ recv_W  [1, D]
            naT_ps = acc.tile([NA, 1], F32, tag="ppn")
            nc.tensor.transpose(out=naT_ps[:], in_=na_row[:], identity=ident[:1, :1])
            na_col = work.tile([NA, 1], F32, tag="na_col")
            nc.vector.tensor_copy(out=na_col[:], in_=naT_ps[:])
            t2_ps = acc.tile([1, D], F32, tag="ppn")
            nc.tensor.matmul(out=t2_ps[:], lhsT=na_col[:], rhs=recv_W_sb[:], start=True, stop=True)
            sin_row = work.tile([1, D], F32, tag="sin_row")
            nc.scalar.activation(out=sin_row[:], in_=t2_ps[:], func=mybir.ActivationFunctionType.Sin)

            # broadcast sin(t2) to all partitions via outer product with ones
            sinb_ps = pp.tile([128, D], F32, tag="pp")
            nc.tensor.matmul(out=sinb_ps[:], lhsT=ones_row[:], rhs=sin_row[:], start=True, stop=True)
            sinb = work.tile([128, D], F32, tag="sinb")
            nc.vector.tensor_copy(out=sinb[:], in_=sinb_ps[:])

            # ---- ue = u + sin(t2); write ue; build ueT; preds ----
            for j in range(NT):
                js = slice(j * 128, (j + 1) * 128)
                nc.vector.tensor_add(out=u_all[:, js], in0=u_all[:, js], in1=sinb[:])
                nc.sync.dma_start(out=ue_out[js, :], in_=u_all[:, js])
                ueT_ps = pp.tile([128, 128], F32, tag="pp")
                nc.tensor.transpose(out=ueT_ps[:], in_=u_all[:, js], identity=ident[:])
                nc.vector.tensor_copy(out=ueT_all[:, js], in_=ueT_ps[:])

            # preds[j*128:(j+1)*128, :] = ue_tile @ item_t shard
            for j in range(NT):
                js = slice(j * 128, (j + 1) * 128)
                orow = outst.tile([128, NI_SH], F32, tag="orow")
                for c in range(N_FULL):
                    cs = slice(c * 512, (c + 1) * 512)
                    p_ps = pp.tile([128, 512], F32, tag="pp")
                    nc.tensor.matmul(
                        out=p_ps[:], lhsT=ueT_all[:, js], rhs=item_t_sb[:, cs],
                        start=True, stop=True,
                    )
                    nc.vector.tensor_copy(out=orow[:, cs], in_=p_ps[:])
                rs = slice(N_FULL * 512, NI_SH)
                p_ps = pp.tile([128, 512], F32, tag="pp")
                nc.tensor.matmul(
                    out=p_ps[:, :N_REM], lhsT=ueT_all[:, js], rhs=item_t_sb[:, rs],
                    start=True, stop=True,
                )
                nc.vector.tensor_copy(out=orow[:, rs], in_=p_ps[:, :N_REM])
                nc.sync.dma_start(out=preds[js, :], in_=orow[:])

            # pos gather (independent; emitted last so it fills idle DSP/DMA time)
            for j in range(NT):
                js = slice(j * 128, (j + 1) * 128)
                pos_t = work.tile([128, D], F32, tag="pos")
                nc.gpsimd.indirect_dma_start(
                    out=pos_t[:],
                    out_offset=None,
                    in_=item_table[:],
                    in_offset=bass.IndirectOffsetOnAxis(ap=pidx_sb[:, j : j + 1], axis=0),
                )
                nc.sync.dma_start(out=pos_out[js, :], in_=pos_t[:])

    nc.finalize()
    return nc


_NC = None


def _get_nc():
    global _NC
    if _NC is None:
        _NC = build_nc()
    return _NC


def _prep_in_maps(inputs):
    inp = {k: np.asarray(v) for k, v in inputs.items()}
    uidx = np.ascontiguousarray(
        inp["user_indices"].astype(np.int32).reshape(NT, 128).T
    )
    pidx = np.ascontiguousarray(
        inp["pos_item_indices"].astype(np.int32).reshape(NT, 128).T
    )
    user_table = np.ascontiguousarray(inp["user_table"], dtype=np.float32)
    item_table = np.ascontiguousarray(inp["item_table"], dtype=np.float32)
    if COMPUTE_BF16:
        import ml_dtypes

        item_T = np.ascontiguousarray(item_table.T.astype(ml_dtypes.bfloat16))  # [D, NI]
    else:
        item_T = np.ascontiguousarray(item_table.T)  # [D, NI]
    send_W = np.ascontiguousarray(inp["send_W"], dtype=np.float32)
    a2 = np.asarray(inp["send_a"], dtype=np.float32)[AD:]
    a2_b = np.ascontiguousarray(np.tile(a2[None, :], (128, 1)))
    anchors_T = np.ascontiguousarray(np.asarray(inp["anchors"], dtype=np.float32).T)
    recv_W = np.ascontiguousarray(inp["recv_W"], dtype=np.float32)
    ln1_g = np.ascontiguousarray(np.tile(np.asarray(inp["ln1_g"], np.float32)[None, :], (128, 1)))
    ln1_b = np.ascontiguousarray(np.tile(np.asarray(inp["ln1_b"], np.float32)[None, :], (128, 1)))
    ln2_g = np.asarray(inp["ln2_g"], np.float32)[None, :]
    ln2_b = np.asarray(inp["ln2_b"], np.float32)[None, :]

    common = dict(
        uidx=uidx, pidx=pidx, user_table=user_table, item_table=item_table,
        send_W=send_W, a2_b=a2_b, anchors_T=anchors_T, recv_W=recv_W,
        ln1_g=ln1_g, ln1_b=ln1_b, ln2_g=ln2_g, ln2_b=ln2_b,
    )
    in_maps = []
    for c in range(NCORES):
        m = dict(common)
        m["item_t"] = np.ascontiguousarray(item_T[:, c * NI_SH : (c + 1) * NI_SH])
        in_maps.append(m)
    return in_maps


def kernel(**inputs):
    nc = _get_nc()
    in_maps = _prep_in_maps(inputs)
    res = run_bass_kernel_spmd(nc, in_maps, core_ids=list(range(NCORES)))
    preds = np.concatenate([res.results[c]["preds"] for c in range(NCORES)], axis=1)
    ue = res.results[0]["ue"]
    pos = res.results[0]["pos"]
    return preds, ue, pos
